# revision 6
# baseline (speedup 1.0000x reference)
"""Trainium2 Bass kernel for the 8-layer position-aware GNN (nn_Net_18915035971604).

Strategy: nodes are bin-packed (<=4 nodes, <=128 edges per bin) and sharded
across 8 cores. Per conv layer: dma_gather of source-node feature rows (bf16,
256B rows) in padded edge order -> per-bin moment matmul on PE with a
block-diagonal basis matrix G (deg_inv folded in) -> second matmul with the
conv weights -> SELU -> transpose -> AllGather of updated node features.
fc1/fc2 run node-sharded with no communication; fc2 writes f32 logits.
"""

import sys
import types

import numpy as np

sys.path.insert(0, "/opt/trn_rl_repo")

import ml_dtypes  # noqa: E402

bf16 = ml_dtypes.bfloat16

N_CORES = 8
N_NODES = 20000
CH = [3, 16, 32, 64, 128, 128, 128, 128, 128]
OUT_CH = 6890
R_FACTOR = np.float32(1.1599 - (-1.2432))
MAXN = 4          # nodes per bin
CAP = 128         # edge rows per bin
GCOLS = MAXN * 10
LAMBDA = float(np.float32(1.0507009873554805))
ALPHA = float(np.float32(1.6732632423543772))
GATHER_CHUNK_BINS = 16  # bins per dma_gather instruction


def _install_hook():
    try:
        from trn_agent_boot.trn_boot import _ntff_profile_via_ctypes
        _hook = _ntff_profile_via_ctypes('/opt/axon/libaxon_pjrt.so')
        mod = types.ModuleType("antenv.axon_hooks")
        mod.get_axon_ntff_profile_hook = lambda: _hook
        mod.set_axon_ntff_profile_hook = lambda h: None
        import antenv
        antenv.axon_hooks = mod
        sys.modules["antenv.axon_hooks"] = mod
    except Exception:
        pass


def _ffd_pack(deg):
    """First-fit-decreasing: bins of <=CAP rows and <=MAXN nodes."""
    order = np.argsort(-deg, kind="stable")
    bins = []
    bin_rows = []
    open_by_rem = {}
    for nd in order:
        d = int(deg[nd])
        placed = False
        for rem in range(max(d, 1), CAP + 1):
            lst = open_by_rem.get(rem)
            if lst:
                bi = lst.pop()
                bins[bi].append(nd)
                bin_rows[bi] += d
                newrem = CAP - bin_rows[bi]
                if len(bins[bi]) < MAXN and newrem > 0:
                    open_by_rem.setdefault(newrem, []).append(bi)
                placed = True
                break
        if not placed:
            bins.append([nd])
            bin_rows.append(d)
            rem0 = CAP - d
            if rem0 > 0:
                open_by_rem.setdefault(rem0, []).append(len(bins) - 1)
    return bins


def _preprocess(x, edge_index, pos):
    src = edge_index[0].astype(np.int64)
    dst = edge_index[1].astype(np.int64)
    E = src.shape[0]
    deg = np.bincount(dst, minlength=N_NODES)
    assert deg.max() <= CAP, "node degree exceeds bin capacity"
    deg_inv = (1.0 / np.maximum(deg, 1.0)).astype(np.float32)

    bins = _ffd_pack(deg)
    nbins = len(bins)
    B = -(-nbins // N_CORES)          # bins per core
    nbins_pad = B * N_CORES
    NSLOT = B * MAXN                  # slots per core
    NSLOT_PAD = -(-NSLOT // 128) * 128
    NROWS = NSLOT_PAD * N_CORES       # global padded node-slot space
    assert NROWS < 32768, "slot ids must fit int16"

    # core c owns bins [c*B, (c+1)*B); node -> (core, slot)
    node_slot = np.full(N_NODES, -1, np.int64)   # global slot id
    for i in range(nbins):
        c, k = divmod(i, B)
        for g, nd in enumerate(bins[i]):
            node_slot[nd] = c * NSLOT_PAD + k * MAXN + g
    assert (node_slot >= 0).all()

    # edges sorted by destination bin/slot; build per-core padded structures
    e_core = np.empty(E, np.int64)
    e_binlocal = np.empty(E, np.int64)
    e_slotg = np.empty(E, np.int64)
    gslot = node_slot[dst]
    e_core = gslot // NSLOT_PAD
    loc = gslot % NSLOT_PAD
    e_binlocal = loc // MAXN
    e_slotg = loc % MAXN

    # basis values with deg_inv folded
    rel = (pos[src] - pos[dst]) / R_FACTOR
    dx, dy, dz = rel[:, 0], rel[:, 1], rel[:, 2]
    basis = np.stack([np.ones_like(dx), dx, dy, dz, dx * dx, dx * dy, dx * dz,
                      dy * dy, dy * dz, dz * dz], axis=1)
    basis *= deg_inv[dst][:, None]

    EPAD = B * CAP
    idx_arrs, g_arrs = [], []
    src_slot = node_slot[src].astype(np.int32)
    for c in range(N_CORES):
        mask = e_core == c
        eb = e_binlocal[mask]
        eg = e_slotg[mask]
        es = src_slot[mask]
        ebas = basis[mask]
        # order edges by (bin, slot)
        order = np.lexsort((eg, eb))
        eb, eg, es, ebas = eb[order], eg[order], es[order], ebas[order]
        # row position within each bin
        rowpos = np.zeros(len(eb), np.int64)
        if len(eb):
            newbin = np.r_[True, eb[1:] != eb[:-1]]
            starts = np.flatnonzero(newbin)
            cum = np.arange(len(eb))
            rowpos = cum - np.repeat(cum[starts], np.diff(np.r_[starts, len(eb)]))
        gidx = eb * CAP + rowpos                     # padded row id in [0, EPAD)
        idx_full = np.zeros(EPAD, np.int32)          # padding rows gather row 0
        idx_full[gidx] = es
        G = np.zeros((EPAD, GCOLS), np.float32)
        G[gidx[:, None], (eg * 10)[:, None] + np.arange(10)[None, :]] = ebas
        # wrap idx into [16, EPAD/16] layout, tile to 128 partitions
        idx16 = idx_full.astype(np.int16).reshape(EPAD // 16, 16).T
        idx_arrs.append(np.tile(idx16, (8, 1)))
        # G dense laid out [128 rows, B*GCOLS]
        g_arrs.append(np.ascontiguousarray(
            G.reshape(B, CAP, GCOLS).transpose(1, 0, 2).reshape(CAP, B * GCOLS)
        ).astype(bf16))

    meta = dict(B=B, NSLOT=NSLOT, NSLOT_PAD=NSLOT_PAD, NROWS=NROWS, EPAD=EPAD,
                node_slot=node_slot)
    return idx_arrs, g_arrs, meta


_BUILD_CACHE = {}


def _build_program(B, NSLOT_PAD, NROWS, EPAD, biases_nonzero):
    import concourse.bacc as bacc
    import concourse.mybir as mybir
    import concourse.tile as tile
    from concourse.masks import make_identity

    key = (B, NSLOT_PAD, NROWS, EPAD, biases_nonzero)
    if key in _BUILD_CACHE:
        return _BUILD_CACHE[key]

    dt = mybir.dt
    AF = mybir.ActivationFunctionType
    OP = mybir.AluOpType

    NL = len(CH) - 1
    nc = bacc.Bacc("TRN2", target_bir_lowering=False, debug=False,
                   num_devices=N_CORES)
    x0_t = nc.dram_tensor("x0", [NROWS, 128], dt.bfloat16, kind="ExternalInput")
    idx_t = nc.dram_tensor("idx", [128, EPAD // 16], dt.int16, kind="ExternalInput")
    g_t = nc.dram_tensor("g", [128, B * GCOLS], dt.bfloat16, kind="ExternalInput")
    wconv_t = nc.dram_tensor("wconv", [128, NL * 10 * 128], dt.bfloat16, kind="ExternalInput")
    fc1w_t = nc.dram_tensor("fc1w", [128, 256], dt.bfloat16, kind="ExternalInput")
    fc2w_t = nc.dram_tensor("fc2w", [128, 2 * OUT_CH], dt.bfloat16, kind="ExternalInput")
    bias_t = nc.dram_tensor("biases", [128, NL + 2], dt.float32, kind="ExternalInput")
    out_t = nc.dram_tensor("out", [NSLOT_PAD, OUT_CH], dt.float32, kind="ExternalOutput")

    CHUNK = GATHER_CHUNK_BINS
    n_gchunks = -(-B // CHUNK)
    SLOTS = B * MAXN                       # logical slot columns (= B*4)
    n_sgroups = -(-SLOTS // 512)

    with tile.TileContext(nc) as tc:
        with (
            tc.tile_pool(name="const", bufs=1) as constp,
            tc.tile_pool(name="resident", bufs=1) as resp,
            tc.tile_pool(name="xg", bufs=2) as xgp,
            tc.tile_pool(name="wstream", bufs=3) as wsp,
            tc.tile_pool(name="ep", bufs=2) as epp,
            tc.tile_pool(name="stage", bufs=1) as stp,
            tc.tile_pool(name="psA", bufs=4, space="PSUM") as psA,
            tc.tile_pool(name="psB", bufs=2, space="PSUM") as psB,
            tc.tile_pool(name="psC", bufs=2, space="PSUM") as psC,
            tc.tile_pool(name="dram", bufs=1, space="DRAM") as dramp,
        ):
            ident = constp.tile([128, 128], dt.bfloat16)
            make_identity(nc, ident[:])
            idx_sb = resp.tile([128, EPAD // 16], dt.int16)
            nc.sync.dma_start(idx_sb[:], idx_t.ap())
            g_sb = resp.tile([128, B * GCOLS], dt.bfloat16)
            nc.sync.dma_start(g_sb[:], g_t.ap())
            fc1_sb = constp.tile([128, 256], dt.bfloat16)
            nc.sync.dma_start(fc1_sb[:], fc1w_t.ap())
            bias_sb = constp.tile([128, NL + 2], dt.float32)
            nc.sync.dma_start(bias_sb[:], bias_t.ap())
            m_sb = resp.tile([128, SLOTS * 10], dt.bfloat16)

            xcur = [x0_t.ap()]  # DRAM source of gathers for current layer

            def psum_copy(i, out_ap, in_ap):
                if i % 2 == 0:
                    nc.scalar.copy(out_ap, in_ap)
                else:
                    nc.vector.tensor_copy(out_ap, in_ap)


            def conv_layer(li):
                cin, cout = CH[li], CH[li + 1]
                w_sb = wsp.tile([128, 10 * 128], dt.bfloat16, tag="wc")
                nc.sync.dma_start(w_sb[:], wconv_t.ap()[:, li * 1280:(li + 1) * 1280])
                # --- first matmul: per-bin moments ---
                for ch in range(n_gchunks):
                    b0 = ch * CHUNK
                    nb = min(CHUNK, B - b0)
                    xg = xgp.tile([128, CHUNK * 128], dt.bfloat16, tag="xg")
                    nc.gpsimd.dma_gather(
                        out_ap=xg[:, :nb * 128].rearrange("p (k e) -> p k e", e=128),
                        in_ap=xcur[0],
                        idxs_ap=idx_sb[:, b0 * 8:(b0 + nb) * 8],
                        num_idxs=nb * 128, num_idxs_reg=nb * 128,
                        elem_size=128, single_packet=False)
                    # 12-bin PSUM groups
                    for p0 in range(0, nb, 12):
                        pn = min(12, nb - p0)
                        ps = psA.tile([128, 12 * GCOLS], dt.float32, tag="mps")
                        for k in range(pn):
                            bk = b0 + p0 + k
                            nc.tensor.matmul(
                                ps[:cin, k * GCOLS:(k + 1) * GCOLS],
                                lhsT=xg[:, (p0 + k) * 128:(p0 + k) * 128 + cin],
                                rhs=g_sb[:, bk * GCOLS:(bk + 1) * GCOLS],
                                start=True, stop=True)
                        psum_copy(p0 // 12,
                            m_sb[:cin, (b0 + p0) * GCOLS:(b0 + p0 + pn) * GCOLS],
                            ps[:cin, :pn * GCOLS])
                # --- second matmul + SELU ---
                m3 = m_sb[:].rearrange("p (s b) -> p s b", b=10)
                h = stp.tile([128, NSLOT_PAD], dt.bfloat16, tag="h")
                for gI in range(n_sgroups):
                    s0 = gI * 512
                    ns = min(512, SLOTS - s0)
                    ps2 = psB.tile([128, 512], dt.float32, tag="zps")
                    for b in range(10):
                        nc.tensor.matmul(
                            ps2[:cout, :ns],
                            lhsT=w_sb[:cin, b * 128:b * 128 + cout],
                            rhs=m3[:cin, s0:s0 + ns, b],
                            start=(b == 0), stop=(b == 9))
                    z = ps2[:cout, :ns]
                    m0 = epp.tile([128, 512], dt.float32, tag="t0")
                    e1 = epp.tile([128, 512], dt.float32, tag="t1")
                    t2 = epp.tile([128, 512], dt.float32, tag="t2")
                    u3 = epp.tile([128, 512], dt.float32, tag="t0")
                    bcol = bias_sb[:cout, li:li + 1]
                    if biases_nonzero:
                        nc.vector.tensor_scalar(m0[:cout, :ns], z, bcol, 0.0, OP.add, OP.min)
                        nc.vector.scalar_tensor_tensor(t2[:cout, :ns], z, bcol, m0[:cout, :ns], OP.add, OP.subtract)
                    else:
                        nc.vector.tensor_scalar(m0[:cout, :ns], z, 0.0, None, OP.min)
                        nc.vector.scalar_tensor_tensor(t2[:cout, :ns], z, 1.0, m0[:cout, :ns], OP.mult, OP.subtract)
                    nc.scalar.activation(e1[:cout, :ns], m0[:cout, :ns], AF.Exp)
                    nc.vector.tensor_scalar(u3[:cout, :ns], e1[:cout, :ns],
                                            LAMBDA * ALPHA, LAMBDA * ALPHA, OP.mult, OP.subtract)
                    nc.vector.scalar_tensor_tensor(h[:cout, s0:s0 + ns], t2[:cout, :ns],
                                                   LAMBDA, u3[:cout, :ns], OP.mult, OP.add)
                if SLOTS < NSLOT_PAD:
                    nc.vector.memset(h[:, SLOTS:NSLOT_PAD], 0.0)
                return h

            for li in range(NL):
                h = conv_layer(li)
                if li == NL - 1:
                    h_last = h
                    break
                # transpose h -> node rows, DMA to slab, AllGather
                xrow = stp.tile([128, NSLOT_PAD], dt.bfloat16, tag="xrow")
                for t in range(NSLOT_PAD // 128):
                    pst = psC.tile([128, 128], dt.bfloat16, tag="tps")
                    nc.tensor.transpose(pst[:], h[:, t * 128:(t + 1) * 128], ident[:])
                    psum_copy(t, xrow[:, t * 128:(t + 1) * 128], pst[:])
                slab = dramp.tile([NSLOT_PAD, 128], dt.bfloat16, tag="slab")
                nc.sync.dma_start(
                    slab[:].rearrange("(t p) f -> p t f", p=128), xrow[:].rearrange("p (t f) -> p t f", f=128))
                xnext = dramp.tile([NROWS, 128], dt.bfloat16, tag=f"xg{li % 2}")
                nc.gpsimd.collective_compute(
                    "AllGather", mybir.AluOpType.bypass,
                    replica_groups=[list(range(N_CORES))],
                    ins=[slab.opt()], outs=[xnext.opt()])
                xcur[0] = xnext[:]

            # --- fc1 ---
            z1h = []
            for hf in range(2):
                z1 = stp.tile([128, NSLOT_PAD], dt.bfloat16, tag=f"z1_{hf}")
                for gI in range(-(-NSLOT_PAD // 512)):
                    s0 = gI * 512
                    ns = min(512, NSLOT_PAD - s0)
                    ps2 = psB.tile([128, 512], dt.float32, tag="zps")
                    nc.tensor.matmul(ps2[:, :ns], lhsT=fc1_sb[:, hf * 128:(hf + 1) * 128],
                                     rhs=h_last[:, s0:s0 + ns], start=True, stop=True)
                    z = ps2[:, :ns]
                    m0 = epp.tile([128, 512], dt.float32, tag="t0")
                    e1 = epp.tile([128, 512], dt.float32, tag="t1")
                    t2 = epp.tile([128, 512], dt.float32, tag="t2")
                    u3 = epp.tile([128, 512], dt.float32, tag="t0")
                    bcol = bias_sb[:, NL + hf:NL + hf + 1]
                    if biases_nonzero:
                        nc.vector.tensor_scalar(m0[:, :ns], z, bcol, 0.0, OP.add, OP.min)
                        nc.vector.scalar_tensor_tensor(t2[:, :ns], z, bcol, m0[:, :ns], OP.add, OP.subtract)
                    else:
                        nc.vector.tensor_scalar(m0[:, :ns], z, 0.0, None, OP.min)
                        nc.vector.scalar_tensor_tensor(t2[:, :ns], z, 1.0, m0[:, :ns], OP.mult, OP.subtract)
                    nc.scalar.activation(e1[:, :ns], m0[:, :ns], AF.Exp)
                    nc.vector.tensor_scalar(u3[:, :ns], e1[:, :ns],
                                            LAMBDA * ALPHA, LAMBDA * ALPHA, OP.mult, OP.subtract)
                    nc.vector.scalar_tensor_tensor(z1[:, s0:s0 + ns], t2[:, :ns],
                                                   LAMBDA, u3[:, :ns], OP.mult, OP.add)
                z1h.append(z1)

            # --- fc2 ---
            OGROUP = 512
            n_og = -(-OUT_CH // OGROUP)
            for og in range(n_og):
                o0 = og * OGROUP
                no = min(OGROUP, OUT_CH - o0)
                w0 = wsp.tile([128, OGROUP], dt.bfloat16, tag="w0")
                w1 = wsp.tile([128, OGROUP], dt.bfloat16, tag="w1")
                nc.sync.dma_start(w0[:, :no], fc2w_t.ap()[:, o0:o0 + no])
                nc.sync.dma_start(w1[:, :no], fc2w_t.ap()[:, OUT_CH + o0:OUT_CH + o0 + no])
                for t in range(NSLOT_PAD // 128):
                    ps2 = psB.tile([128, 512], dt.float32, tag="zps")
                    nc.tensor.matmul(ps2[:, :no], lhsT=z1h[0][:, t * 128:(t + 1) * 128],
                                     rhs=w0[:, :no], start=True, stop=False)
                    nc.tensor.matmul(ps2[:, :no], lhsT=z1h[1][:, t * 128:(t + 1) * 128],
                                     rhs=w1[:, :no], start=False, stop=True)
                    ob = epp.tile([128, OGROUP], dt.float32, tag="t2")
                    psum_copy(t, ob[:, :no], ps2[:, :no])
                    nc.sync.dma_start(out_t.ap()[t * 128:(t + 1) * 128, o0:o0 + no], ob[:, :no])

    nc.compile()
    _BUILD_CACHE[key] = nc
    return nc


def kernel(x, edge_index, pos, conv_Ws, conv_bs, fc1_W, fc1_b, fc2_W, fc2_b):
    _install_hook()
    from concourse.bass_utils import run_bass_kernel_spmd

    x = np.asarray(x, np.float32)
    pos = np.asarray(pos, np.float32)
    edge_index = np.asarray(edge_index)
    conv_Ws = [np.asarray(w, np.float32) for w in conv_Ws]
    conv_bs = [np.asarray(b, np.float32) for b in conv_bs]
    fc1_W = np.asarray(fc1_W, np.float32)
    fc1_b = np.asarray(fc1_b, np.float32)
    fc2_W = np.asarray(fc2_W, np.float32)
    fc2_b = np.asarray(fc2_b, np.float32)

    idx_arrs, g_arrs, meta = _preprocess(x, edge_index, pos)
    B, NSLOT_PAD, NROWS, EPAD = meta["B"], meta["NSLOT_PAD"], meta["NROWS"], meta["EPAD"]
    node_slot = meta["node_slot"]
    NL = len(CH) - 1

    biases_nonzero = any(np.any(b != 0) for b in conv_bs) or np.any(fc1_b != 0)

    # X0: node features in slot rows
    X0 = np.zeros((NROWS, 128), bf16)
    X0[node_slot, :3] = x.astype(bf16)

    # conv weights [cin, 10*128] per layer, concatenated
    wconv = np.zeros((128, NL * 10 * 128), bf16)
    for li in range(NL):
        cin, cout = CH[li], CH[li + 1]
        w = conv_Ws[li]  # [10, cin, cout]
        for b in range(10):
            wconv[:cin, (li * 10 + b) * 128:(li * 10 + b) * 128 + cout] = w[b].astype(bf16)
    fc1w = fc1_W.astype(bf16)                      # [128, 256]
    fc2w = np.concatenate([fc2_W[:128], fc2_W[128:]], axis=1).astype(bf16)  # [128, 2*OUT_CH]
    biases = np.zeros((128, NL + 2), np.float32)
    for li in range(NL):
        biases[:CH[li + 1], li] = conv_bs[li]
    biases[:, NL] = fc1_b[:128]
    biases[:, NL + 1] = fc1_b[128:]

    nc = _build_program(B, NSLOT_PAD, NROWS, EPAD, bool(biases_nonzero))

    in_maps = []
    for c in range(N_CORES):
        in_maps.append({
            "x0": X0, "idx": idx_arrs[c], "g": g_arrs[c], "wconv": wconv,
            "fc1w": fc1w, "fc2w": fc2w, "biases": biases,
        })
    import os
    trace = bool(os.environ.get("KERNEL_TRACE"))
    res = run_bass_kernel_spmd(nc, in_maps, core_ids=list(range(N_CORES)), trace=trace)
    if trace:
        print("HW exec time: %d ns" % (res.exec_time_ns or -1))

    out = np.empty((N_NODES, OUT_CH), np.float32)
    full = np.concatenate([res.results[c]["out"] for c in range(N_CORES)], axis=0)
    # full is [N_CORES*NSLOT_PAD, OUT_CH] in global-slot order
    out[:] = full[node_slot]
    if np.any(fc2_b != 0):
        out += fc2_b[None, :]
    return out


# revision 9
# speedup vs baseline: 1.0112x; 1.0112x over previous
"""Trainium2 Bass kernel for the 8-layer position-aware GNN (nn_Net_18915035971604).

Strategy: nodes are bin-packed (<=4 nodes, <=128 edges per bin) and sharded
across 8 cores. Per conv layer: dma_gather of source-node feature rows (bf16,
256B rows) in padded edge order -> per-bin moment matmul on PE with a
block-diagonal basis matrix G (deg_inv folded in) -> second matmul with the
conv weights -> SELU -> transpose -> AllGather of updated node features.
fc1/fc2 run node-sharded with no communication; fc2 writes f32 logits.
"""

import sys
import types

import numpy as np

sys.path.insert(0, "/opt/trn_rl_repo")

import ml_dtypes  # noqa: E402

bf16 = ml_dtypes.bfloat16

N_CORES = 8
N_NODES = 20000
CH = [3, 16, 32, 64, 128, 128, 128, 128, 128]
OUT_CH = 6890
R_FACTOR = np.float32(1.1599 - (-1.2432))
MAXN = 4          # nodes per bin
CAP = 128         # edge rows per bin
GCOLS = MAXN * 10
LAMBDA = float(np.float32(1.0507009873554805))
ALPHA = float(np.float32(1.6732632423543772))
GATHER_CHUNK_BINS = 16  # bins per dma_gather instruction


def _install_hook():
    try:
        from trn_agent_boot.trn_boot import _ntff_profile_via_ctypes
        _hook = _ntff_profile_via_ctypes('/opt/axon/libaxon_pjrt.so')
        mod = types.ModuleType("antenv.axon_hooks")
        mod.get_axon_ntff_profile_hook = lambda: _hook
        mod.set_axon_ntff_profile_hook = lambda h: None
        import antenv
        antenv.axon_hooks = mod
        sys.modules["antenv.axon_hooks"] = mod
    except Exception:
        pass


def _ffd_pack(deg):
    """First-fit-decreasing: bins of <=CAP rows and <=MAXN nodes."""
    order = np.argsort(-deg, kind="stable")
    bins = []
    bin_rows = []
    open_by_rem = {}
    for nd in order:
        d = int(deg[nd])
        placed = False
        for rem in range(max(d, 1), CAP + 1):
            lst = open_by_rem.get(rem)
            if lst:
                bi = lst.pop()
                bins[bi].append(nd)
                bin_rows[bi] += d
                newrem = CAP - bin_rows[bi]
                if len(bins[bi]) < MAXN and newrem > 0:
                    open_by_rem.setdefault(newrem, []).append(bi)
                placed = True
                break
        if not placed:
            bins.append([nd])
            bin_rows.append(d)
            rem0 = CAP - d
            if rem0 > 0:
                open_by_rem.setdefault(rem0, []).append(len(bins) - 1)
    return bins


def _preprocess(x, edge_index, pos):
    src = edge_index[0].astype(np.int64)
    dst = edge_index[1].astype(np.int64)
    E = src.shape[0]
    deg = np.bincount(dst, minlength=N_NODES)
    assert deg.max() <= CAP, "node degree exceeds bin capacity"
    deg_inv = (1.0 / np.maximum(deg, 1.0)).astype(np.float32)

    bins = _ffd_pack(deg)
    nbins = len(bins)
    B = -(-nbins // N_CORES)          # bins per core
    nbins_pad = B * N_CORES
    NSLOT = B * MAXN                  # slots per core
    NSLOT_PAD = -(-NSLOT // 128) * 128
    NROWS = NSLOT_PAD * N_CORES       # global padded node-slot space
    assert NROWS < 32768, "slot ids must fit int16"

    # core c owns bins [c*B, (c+1)*B); node -> (core, slot)
    node_slot = np.full(N_NODES, -1, np.int64)   # global slot id
    for i in range(nbins):
        c, k = divmod(i, B)
        for g, nd in enumerate(bins[i]):
            node_slot[nd] = c * NSLOT_PAD + k * MAXN + g
    assert (node_slot >= 0).all()

    # edges sorted by destination bin/slot; build per-core padded structures
    e_core = np.empty(E, np.int64)
    e_binlocal = np.empty(E, np.int64)
    e_slotg = np.empty(E, np.int64)
    gslot = node_slot[dst]
    e_core = gslot // NSLOT_PAD
    loc = gslot % NSLOT_PAD
    e_binlocal = loc // MAXN
    e_slotg = loc % MAXN

    # basis values with deg_inv folded
    rel = (pos[src] - pos[dst]) / R_FACTOR
    dx, dy, dz = rel[:, 0], rel[:, 1], rel[:, 2]
    basis = np.stack([np.ones_like(dx), dx, dy, dz, dx * dx, dx * dy, dx * dz,
                      dy * dy, dy * dz, dz * dz], axis=1)
    basis *= deg_inv[dst][:, None]

    EPAD = B * CAP
    idx_arrs, g_arrs = [], []
    src_slot = node_slot[src].astype(np.int32)
    for c in range(N_CORES):
        mask = e_core == c
        eb = e_binlocal[mask]
        eg = e_slotg[mask]
        es = src_slot[mask]
        ebas = basis[mask]
        # order edges by (bin, slot)
        order = np.lexsort((eg, eb))
        eb, eg, es, ebas = eb[order], eg[order], es[order], ebas[order]
        # row position within each bin
        rowpos = np.zeros(len(eb), np.int64)
        if len(eb):
            newbin = np.r_[True, eb[1:] != eb[:-1]]
            starts = np.flatnonzero(newbin)
            cum = np.arange(len(eb))
            rowpos = cum - np.repeat(cum[starts], np.diff(np.r_[starts, len(eb)]))
        gidx = eb * CAP + rowpos                     # padded row id in [0, EPAD)
        idx_full = np.zeros(EPAD, np.int32)          # padding rows gather row 0
        idx_full[gidx] = es
        G = np.zeros((EPAD, GCOLS), np.float32)
        G[gidx[:, None], (eg * 10)[:, None] + np.arange(10)[None, :]] = ebas
        # wrap idx into [16, EPAD/16] layout, tile to 128 partitions
        idx16 = idx_full.astype(np.int16).reshape(EPAD // 16, 16).T
        idx_arrs.append(np.tile(idx16, (8, 1)))
        # G dense laid out [128 rows, B*GCOLS]
        g_arrs.append(np.ascontiguousarray(
            G.reshape(B, CAP, GCOLS).transpose(1, 0, 2).reshape(CAP, B * GCOLS)
        ).astype(bf16))

    meta = dict(B=B, NSLOT=NSLOT, NSLOT_PAD=NSLOT_PAD, NROWS=NROWS, EPAD=EPAD,
                node_slot=node_slot)
    return idx_arrs, g_arrs, meta


_BUILD_CACHE = {}


def _build_program(B, NSLOT_PAD, NROWS, EPAD, biases_nonzero):
    import concourse.bacc as bacc
    import concourse.mybir as mybir
    import concourse.tile as tile
    from concourse.masks import make_identity

    key = (B, NSLOT_PAD, NROWS, EPAD, biases_nonzero)
    if key in _BUILD_CACHE:
        return _BUILD_CACHE[key]

    dt = mybir.dt
    AF = mybir.ActivationFunctionType
    OP = mybir.AluOpType

    NL = len(CH) - 1
    nc = bacc.Bacc("TRN2", target_bir_lowering=False, debug=False,
                   num_devices=N_CORES)
    x0_t = nc.dram_tensor("x0", [NROWS, 128], dt.bfloat16, kind="ExternalInput")
    idx_t = nc.dram_tensor("idx", [128, EPAD // 16], dt.int16, kind="ExternalInput")
    g_t = nc.dram_tensor("g", [128, B * GCOLS], dt.bfloat16, kind="ExternalInput")
    wconv_t = nc.dram_tensor("wconv", [128, NL * 10 * 128], dt.bfloat16, kind="ExternalInput")
    fc1w_t = nc.dram_tensor("fc1w", [128, 256], dt.bfloat16, kind="ExternalInput")
    fc2w_t = nc.dram_tensor("fc2w", [128, 2 * OUT_CH], dt.bfloat16, kind="ExternalInput")
    bias_t = nc.dram_tensor("biases", [128, NL + 2], dt.float32, kind="ExternalInput")
    out_t = nc.dram_tensor("out", [NSLOT_PAD, OUT_CH], dt.float32, kind="ExternalOutput")

    CHUNK = GATHER_CHUNK_BINS
    n_gchunks = -(-B // CHUNK)
    SLOTS = B * MAXN                       # logical slot columns (= B*4)
    n_sgroups = -(-SLOTS // 512)

    with tile.TileContext(nc) as tc:
        with (
            tc.tile_pool(name="const", bufs=1) as constp,
            tc.tile_pool(name="resident", bufs=1) as resp,
            tc.tile_pool(name="xg", bufs=2) as xgp,
            tc.tile_pool(name="wstream", bufs=3) as wsp,
            tc.tile_pool(name="ep", bufs=2) as epp,
            tc.tile_pool(name="stage", bufs=1) as stp,
            tc.tile_pool(name="psA", bufs=4, space="PSUM") as psA,
            tc.tile_pool(name="psB", bufs=2, space="PSUM") as psB,
            tc.tile_pool(name="psC", bufs=2, space="PSUM") as psC,
            tc.tile_pool(name="dram", bufs=1, space="DRAM") as dramp,
        ):
            ident = constp.tile([128, 128], dt.bfloat16)
            make_identity(nc, ident[:])
            idx_sb = resp.tile([128, EPAD // 16], dt.int16)
            nc.sync.dma_start(idx_sb[:], idx_t.ap())
            g_sb = resp.tile([128, B * GCOLS], dt.bfloat16)
            nc.sync.dma_start(g_sb[:], g_t.ap())
            fc1_sb = constp.tile([128, 256], dt.bfloat16)
            nc.sync.dma_start(fc1_sb[:], fc1w_t.ap())
            bias_sb = constp.tile([128, NL + 2], dt.float32)
            nc.sync.dma_start(bias_sb[:], bias_t.ap())
            n_mg = -(-B // 128)
            m_tiles = [resp.tile([128, min(128, B - 128 * g) * GCOLS], dt.bfloat16,
                                 name=f"mtile{g}", tag=f"m{g}")
                       for g in range(n_mg)]

            xcur = [x0_t.ap()]  # DRAM source of gathers for current layer

            def psum_copy(i, out_ap, in_ap):
                if i % 2 == 0:
                    nc.scalar.copy(out_ap, in_ap)
                else:
                    nc.vector.tensor_copy(out_ap, in_ap)


            def conv_layer(li):
                cin, cout = CH[li], CH[li + 1]
                w_sb = wsp.tile([128, 10 * 128], dt.bfloat16, tag="wc")
                nc.sync.dma_start(w_sb[:], wconv_t.ap()[:, li * 1280:(li + 1) * 1280])
                # --- first matmul: per-bin moments ---
                for ch in range(n_gchunks):
                    b0 = ch * CHUNK
                    nb = min(CHUNK, B - b0)
                    xg = xgp.tile([128, CHUNK * 128], dt.bfloat16, tag="xg")
                    nc.gpsimd.dma_gather(
                        out_ap=xg[:, :nb * 128].rearrange("p (k e) -> p k e", e=128),
                        in_ap=xcur[0],
                        idxs_ap=idx_sb[:, b0 * 8:(b0 + nb) * 8],
                        num_idxs=nb * 128, num_idxs_reg=nb * 128,
                        elem_size=128, single_packet=False)
                    # 8-bin PSUM groups (aligned with 128-bin M groups)
                    for p0 in range(0, nb, 8):
                        pn = min(8, nb - p0)
                        ps = psA.tile([128, 8 * GCOLS], dt.float32, tag="mps")
                        for k in range(pn):
                            bk = b0 + p0 + k
                            nc.tensor.matmul(
                                ps[:cin, k * GCOLS:(k + 1) * GCOLS],
                                lhsT=xg[:, (p0 + k) * 128:(p0 + k) * 128 + cin],
                                rhs=g_sb[:, bk * GCOLS:(bk + 1) * GCOLS],
                                start=True, stop=True)
                        babs = b0 + p0
                        mg, moff = babs // 128, babs % 128
                        psum_copy(p0 // 8,
                            m_tiles[mg][:cin, moff * GCOLS:(moff + pn) * GCOLS],
                            ps[:cin, :pn * GCOLS])
                # --- second matmul + SELU ---
                h = stp.tile([128, NSLOT_PAD], dt.bfloat16, tag="h")
                for gI in range(n_sgroups):
                    s0 = gI * 512
                    ns = min(512, SLOTS - s0)
                    m3 = m_tiles[gI][:].rearrange("p (s b) -> p s b", b=10)
                    ps2 = psB.tile([128, 512], dt.float32, tag="zps")
                    for b in range(10):
                        nc.tensor.matmul(
                            ps2[:cout, :ns],
                            lhsT=w_sb[:cin, b * 128:b * 128 + cout],
                            rhs=m3[:cin, :ns, b],
                            start=(b == 0), stop=(b == 9))
                    z = ps2[:cout, :ns]
                    m0 = epp.tile([128, 512], dt.float32, tag="t0")
                    e1 = epp.tile([128, 512], dt.float32, tag="t1")
                    t2 = epp.tile([128, 512], dt.float32, tag="t2")
                    u3 = epp.tile([128, 512], dt.float32, tag="t0")
                    bcol = bias_sb[:cout, li:li + 1]
                    if biases_nonzero:
                        nc.vector.tensor_scalar(m0[:cout, :ns], z, bcol, 0.0, OP.add, OP.min)
                        nc.vector.scalar_tensor_tensor(t2[:cout, :ns], z, bcol, m0[:cout, :ns], OP.add, OP.subtract)
                    else:
                        nc.vector.tensor_scalar(m0[:cout, :ns], z, 0.0, None, OP.min)
                        nc.vector.scalar_tensor_tensor(t2[:cout, :ns], z, 1.0, m0[:cout, :ns], OP.mult, OP.subtract)
                    nc.scalar.activation(e1[:cout, :ns], m0[:cout, :ns], AF.Exp)
                    nc.vector.tensor_scalar(u3[:cout, :ns], e1[:cout, :ns],
                                            LAMBDA * ALPHA, LAMBDA * ALPHA, OP.mult, OP.subtract)
                    nc.vector.scalar_tensor_tensor(h[:cout, s0:s0 + ns], t2[:cout, :ns],
                                                   LAMBDA, u3[:cout, :ns], OP.mult, OP.add)
                if SLOTS < NSLOT_PAD:
                    nc.vector.memset(h[:, SLOTS:NSLOT_PAD], 0.0)
                return h

            for li in range(NL):
                h = conv_layer(li)
                if li == NL - 1:
                    h_last = h
                    break
                # transpose h -> node rows, DMA to slab, AllGather
                xrow = stp.tile([128, NSLOT_PAD], dt.bfloat16, tag="xrow")
                for t in range(NSLOT_PAD // 128):
                    pst = psC.tile([128, 128], dt.bfloat16, tag="tps")
                    nc.tensor.transpose(pst[:], h[:, t * 128:(t + 1) * 128], ident[:])
                    psum_copy(t, xrow[:, t * 128:(t + 1) * 128], pst[:])
                slab = dramp.tile([NSLOT_PAD, 128], dt.bfloat16, tag="slab")
                nc.sync.dma_start(
                    slab[:].rearrange("(t p) f -> p t f", p=128), xrow[:].rearrange("p (t f) -> p t f", f=128))
                xnext = dramp.tile([NROWS, 128], dt.bfloat16, tag=f"xg{li % 2}")
                nc.gpsimd.collective_compute(
                    "AllGather", mybir.AluOpType.bypass,
                    replica_groups=[list(range(N_CORES))],
                    ins=[slab.opt()], outs=[xnext.opt()])
                xcur[0] = xnext[:]

            # --- fc1 ---
            z1h = []
            for hf in range(2):
                z1 = stp.tile([128, NSLOT_PAD], dt.bfloat16, tag=f"z1_{hf}")
                for gI in range(-(-NSLOT_PAD // 512)):
                    s0 = gI * 512
                    ns = min(512, NSLOT_PAD - s0)
                    ps2 = psB.tile([128, 512], dt.float32, tag="zps")
                    nc.tensor.matmul(ps2[:, :ns], lhsT=fc1_sb[:, hf * 128:(hf + 1) * 128],
                                     rhs=h_last[:, s0:s0 + ns], start=True, stop=True)
                    z = ps2[:, :ns]
                    m0 = epp.tile([128, 512], dt.float32, tag="t0")
                    e1 = epp.tile([128, 512], dt.float32, tag="t1")
                    t2 = epp.tile([128, 512], dt.float32, tag="t2")
                    u3 = epp.tile([128, 512], dt.float32, tag="t0")
                    bcol = bias_sb[:, NL + hf:NL + hf + 1]
                    if biases_nonzero:
                        nc.vector.tensor_scalar(m0[:, :ns], z, bcol, 0.0, OP.add, OP.min)
                        nc.vector.scalar_tensor_tensor(t2[:, :ns], z, bcol, m0[:, :ns], OP.add, OP.subtract)
                    else:
                        nc.vector.tensor_scalar(m0[:, :ns], z, 0.0, None, OP.min)
                        nc.vector.scalar_tensor_tensor(t2[:, :ns], z, 1.0, m0[:, :ns], OP.mult, OP.subtract)
                    nc.scalar.activation(e1[:, :ns], m0[:, :ns], AF.Exp)
                    nc.vector.tensor_scalar(u3[:, :ns], e1[:, :ns],
                                            LAMBDA * ALPHA, LAMBDA * ALPHA, OP.mult, OP.subtract)
                    nc.vector.scalar_tensor_tensor(z1[:, s0:s0 + ns], t2[:, :ns],
                                                   LAMBDA, u3[:, :ns], OP.mult, OP.add)
                z1h.append(z1)

            # --- fc2 ---
            OGROUP = 512
            n_og = -(-OUT_CH // OGROUP)
            for og in range(n_og):
                o0 = og * OGROUP
                no = min(OGROUP, OUT_CH - o0)
                w0 = wsp.tile([128, OGROUP], dt.bfloat16, tag="w0")
                w1 = wsp.tile([128, OGROUP], dt.bfloat16, tag="w1")
                nc.sync.dma_start(w0[:, :no], fc2w_t.ap()[:, o0:o0 + no])
                nc.sync.dma_start(w1[:, :no], fc2w_t.ap()[:, OUT_CH + o0:OUT_CH + o0 + no])
                for t in range(NSLOT_PAD // 128):
                    ps2 = psB.tile([128, 512], dt.float32, tag="zps")
                    nc.tensor.matmul(ps2[:, :no], lhsT=z1h[0][:, t * 128:(t + 1) * 128],
                                     rhs=w0[:, :no], start=True, stop=False)
                    nc.tensor.matmul(ps2[:, :no], lhsT=z1h[1][:, t * 128:(t + 1) * 128],
                                     rhs=w1[:, :no], start=False, stop=True)
                    ob = epp.tile([128, OGROUP], dt.float32, tag="t2")
                    psum_copy(t, ob[:, :no], ps2[:, :no])
                    nc.sync.dma_start(out_t.ap()[t * 128:(t + 1) * 128, o0:o0 + no], ob[:, :no])

    nc.compile()
    _BUILD_CACHE[key] = nc
    return nc


def kernel(x, edge_index, pos, conv_Ws, conv_bs, fc1_W, fc1_b, fc2_W, fc2_b):
    _install_hook()
    from concourse.bass_utils import run_bass_kernel_spmd

    x = np.asarray(x, np.float32)
    pos = np.asarray(pos, np.float32)
    edge_index = np.asarray(edge_index)
    conv_Ws = [np.asarray(w, np.float32) for w in conv_Ws]
    conv_bs = [np.asarray(b, np.float32) for b in conv_bs]
    fc1_W = np.asarray(fc1_W, np.float32)
    fc1_b = np.asarray(fc1_b, np.float32)
    fc2_W = np.asarray(fc2_W, np.float32)
    fc2_b = np.asarray(fc2_b, np.float32)

    idx_arrs, g_arrs, meta = _preprocess(x, edge_index, pos)
    B, NSLOT_PAD, NROWS, EPAD = meta["B"], meta["NSLOT_PAD"], meta["NROWS"], meta["EPAD"]
    node_slot = meta["node_slot"]
    NL = len(CH) - 1

    biases_nonzero = any(np.any(b != 0) for b in conv_bs) or np.any(fc1_b != 0)

    # X0: node features in slot rows
    X0 = np.zeros((NROWS, 128), bf16)
    X0[node_slot, :3] = x.astype(bf16)

    # conv weights [cin, 10*128] per layer, concatenated
    wconv = np.zeros((128, NL * 10 * 128), bf16)
    for li in range(NL):
        cin, cout = CH[li], CH[li + 1]
        w = conv_Ws[li]  # [10, cin, cout]
        for b in range(10):
            wconv[:cin, (li * 10 + b) * 128:(li * 10 + b) * 128 + cout] = w[b].astype(bf16)
    fc1w = fc1_W.astype(bf16)                      # [128, 256]
    fc2w = np.concatenate([fc2_W[:128], fc2_W[128:]], axis=1).astype(bf16)  # [128, 2*OUT_CH]
    biases = np.zeros((128, NL + 2), np.float32)
    for li in range(NL):
        biases[:CH[li + 1], li] = conv_bs[li]
    biases[:, NL] = fc1_b[:128]
    biases[:, NL + 1] = fc1_b[128:]

    nc = _build_program(B, NSLOT_PAD, NROWS, EPAD, bool(biases_nonzero))

    in_maps = []
    for c in range(N_CORES):
        in_maps.append({
            "x0": X0, "idx": idx_arrs[c], "g": g_arrs[c], "wconv": wconv,
            "fc1w": fc1w, "fc2w": fc2w, "biases": biases,
        })
    import os
    trace = bool(os.environ.get("KERNEL_TRACE"))
    res = run_bass_kernel_spmd(nc, in_maps, core_ids=list(range(N_CORES)), trace=trace)
    if trace:
        print("HW exec time: %d ns" % (res.exec_time_ns or -1))

    out = np.empty((N_NODES, OUT_CH), np.float32)
    full = np.concatenate([res.results[c]["out"] for c in range(N_CORES)], axis=0)
    # full is [N_CORES*NSLOT_PAD, OUT_CH] in global-slot order
    out[:] = full[node_slot]
    if np.any(fc2_b != 0):
        out += fc2_b[None, :]
    return out
